# revision 58
# baseline (speedup 1.0000x reference)
"""Trainium2 Bass kernel for nn_BatchMultiHeadGraphAttention (v3).

Math: out[b,c,h] = softmax_j(mask(leaky(src_i + dst_j))) @ Hm  where
Hm = h[b,c] @ w[c,h], t = tanh(Hm), src = t @ a_src, dst = t @ a_dst.

Identity: exp(leaky(x)) = max(e^x, e^{0.2x}), both branches rank-1 in
(i,j).  With s_ij = 1{src_i >= -dst_j}, Vp = V .* s (V = adj+selfloops):
  num = Vp @ (b .* Haug) + r_i * ( V @ (d .* Haug) - Vp @ (d .* Haug) )
  b_j = e^{dst_j}, d_j = e^{0.2 dst_j}, r_i = e^{-0.8 src_i},
  Haug = [Hm | 1];  out = num[:, :64] / num[:, 64].

v6 (132.9us, from 144.9us baseline): negated-ov injected into the
d-columns of the Vp PSUM while each slot's accumulation group is still
the bank's current one (PSUM accumulate groups are per-bank: an
out-of-group start=False matmul OVERWRITES), killing the rov staging;
combine = per-ib ACT u=r*(ppd-ov) + one batched vector TT per
half-head over a 4-slot padded PSUM tile (-num; sign cancels in
num/den); batched attention-vector exps via strided APs; Hbd scaled
via broadcast TTs on gpsimd batched over heads; depth-2 mask prefetch
interleaved with the combine TTs on the in-order vector queue;
per-half reciprocal/divide/DMA drain.

Engine constraints learned: Pool(gpsimd) has no TensorScalarPtr ops,
no PSUM access, no is_ge; DVE STT supports no 2x perf modes (4 memory
streams); tensor_mask/tensor_paged_mask stock encoders crash the
device; DVE ops can read at most one PSUM operand.

Sharding: core = b*2 + cpair; each core does one b and two c's (4 heads).
"""

import os
import sys
from contextlib import ExitStack

import numpy as np
import ml_dtypes

sys.path.insert(0, "/opt/trn_rl_repo")

import concourse.bass as bass
import concourse.bacc as bacc
import concourse.tile as tile
from concourse import mybir
from concourse.masks import make_identity
from concourse.bass_utils import run_bass_kernel_spmd

F32 = mybir.dt.float32
BF16 = mybir.dt.bfloat16
AF = mybir.ActivationFunctionType
OP = mybir.AluOpType

N = 1024
NB = 8
F = 64
C2 = 2
NH = 4

# tuning knobs (gpsimd/Pool cannot run TensorScalarPtr ops: no TSP/STT there)
MASK_NG = int(os.environ.get("MASK_NG", "0"))      # jb's per head on gpsimd
COMBINE_ENGINE = os.environ.get("COMBINE_ENGINE", "gpsimd")
STAGE_ENGINE = os.environ.get("STAGE_ENGINE", "gpsimd")
HBD_B_ENGINE = os.environ.get("HBD_B_ENGINE", "gpsimd")
HBD_D_ENGINE = os.environ.get("HBD_D_ENGINE", "gpsimd")


def build_kernel(nc: bass.Bass, tc: tile.TileContext, ctx: ExitStack, ins, out_ap):
    vT_ap = ins["vT"]
    hTe_ap = ins["hTe"]
    we_ap = ins["we"]
    wb_ap = ins["wb"]
    aab_ap = ins["aab"]

    # ---------------- pools ----------------
    constp = ctx.enter_context(tc.tile_pool(name="const", bufs=1))
    apool = ctx.enter_context(tc.tile_pool(name="apool", bufs=1))
    vppool = ctx.enter_context(tc.tile_pool(name="vppool", bufs=4))
    smallp = ctx.enter_context(tc.tile_pool(name="smallp", bufs=2))
    vecp = ctx.enter_context(tc.tile_pool(name="vecp", bufs=1))
    sbcp = ctx.enter_context(tc.tile_pool(name="sbcp", bufs=8))
    pshm = ctx.enter_context(tc.tile_pool(name="pshm", bufs=1, space="PSUM"))
    psvec = ctx.enter_context(tc.tile_pool(name="psvec", bufs=1, space="PSUM"))
    psov = ctx.enter_context(tc.tile_pool(name="psov", bufs=2, space="PSUM"))
    # half-head PSUM: 4 ib-slots of 256 f32 (padded to avoid bank straddle)
    pspp = ctx.enter_context(tc.tile_pool(name="pspp", bufs=2, space="PSUM"))
    dramp = ctx.enter_context(tc.tile_pool(name="dramp", bufs=1, space="DRAM"))

    # ---------------- constants (small params first, vT after) ----------
    vT = constp.tile([128, NB, N], BF16)
    hTe = constp.tile([65, 2, C2, 512], BF16)  # [65, nh, c, 512]
    wb = constp.tile([64, C2, NH, F], BF16)
    aab = constp.tile([128, C2, 2, 4], BF16)
    we = constp.tile([65, C2, NH * 65], BF16)
    nc.sync.dma_start(out=wb[:], in_=wb_ap[:])
    nc.sync.dma_start(out=hTe[:, 0], in_=hTe_ap[0])
    nc.sync.dma_start(out=hTe[:, 1], in_=hTe_ap[1])
    nc.sync.dma_start(out=aab[:], in_=aab_ap[:])
    nc.sync.dma_start(out=we[:], in_=we_ap[:])
    for jb in range(NB):
        nc.sync.dma_start(out=vT[:, jb, :], in_=vT_ap[jb * 128:(jb + 1) * 128, :])

    # ---------------- persistent A-stage outputs ----------------
    H_aug = apool.tile([128, C2, NB, NH, 65], BF16)
    Hbd = apool.tile([128, C2, NB, NH, 130], BF16)
    srcb_l = [apool.tile([4, 2, N], BF16, tag=f"srcb{i}", name=f"srcb{i}")
              for i in range(C2)]
    srd = dramp.tile([C2, 4, 2, N], BF16, tag="srd")
    # svl[:, c, hp, nb, k]: k = (src_h0, negdst_h0, src_h1, negdst_h1)
    svl = apool.tile([128, C2, 2, NB, 4], F32, tag="svl")
    # rr[:, c, hp, hr, nb] = e^{-0.8 src} for head h = 2*hp + hr
    rr = apool.tile([128, C2, 2, 2, NB], F32, tag="rr")
    # ovn[:, c, ib, h, :] = -(V @ (d .* Haug)) staged bf16
    ovn = apool.tile([128, C2, NB, NH, 65], BF16, tag="ovn")
    ident = constp.tile([128, 128], BF16)
    make_identity(nc, ident[:])

    sbc_l = {}
    vpt_l = {}

    def make_sbc(gh):
        c, h = gh // NH, gh % NH
        hp, hr = h // 2, h % 2
        sbc = sbcp.tile([128, N], BF16, tag="sbc")
        nc.sync.dma_start(
            out=sbc[:],
            in_=srd[c, 2 * hr:2 * hr + 1, hp, :].to_broadcast([128, N]),
        )
        sbc_l[gh] = sbc

    eng = {"vector": nc.vector, "gpsimd": nc.gpsimd}

    # =================== stage A (both c) ===================
    for c in range(C2):
        # ---- A2: tTb = tanh(Hm).T per head pair [128, hp, N] bf16 ----
        tTb = smallp.tile([128, 2, N], BF16, tag="ttb")
        for hp in range(2):
            for nh in range(2):
                pht = pshm.tile([128, 512], F32, tag="ph")
                nc.tensor.matmul(
                    pht[:],
                    lhsT=wb[:, c, 2 * hp:2 * hp + 2, :],
                    rhs=hTe[0:64, nh, c, :],
                    start=True,
                    stop=True,
                )
                nc.scalar.activation(
                    out=tTb[:, hp, nh * 512:(nh + 1) * 512], in_=pht[:],
                    func=AF.Tanh,
                )

        # ---- A3: attention vectors in column layout ----
        psv = psvec.tile([128, 2, NB, 4], F32, tag="psv")
        for hp in range(2):
            for nb in range(NB):
                nc.tensor.matmul(
                    psv[:, hp, nb, :],
                    lhsT=tTb[:, hp, nb * 128:(nb + 1) * 128],
                    rhs=aab[:, c, hp, :],
                    start=True,
                    stop=True,
                )
        nc.scalar.activation(out=svl[:, c], in_=psv[:], func=AF.Copy)

        # batched per-head exps; (hp, hr, nb) AP order == h-major layout
        sv_k = svl[:, c].rearrange("p hp nb (hr two) -> p hp hr nb two", hr=2)
        src_ap = sv_k[:, :, :, :, 0]
        dst_ap = sv_k[:, :, :, :, 1]
        nc.scalar.activation(
            out=rr[:, c], in_=src_ap, func=AF.Exp, scale=-0.8,
        )
        bcol = vecp.tile([128, 2, 2, NB], F32, tag="bcol")
        dcol = vecp.tile([128, 2, 2, NB], F32, tag="dcol")
        nc.scalar.activation(out=bcol[:], in_=dst_ap, func=AF.Exp, scale=-1.0)
        nc.scalar.activation(out=dcol[:], in_=dst_ap, func=AF.Exp, scale=-0.2)

        # ---- A4: src row layout -> DRAM (for free-dim broadcast) ----
        srcb = srcb_l[c]
        for hp in range(2):
            for nh in range(2):
                psr = psvec.tile([4, 512], F32, tag="psv")
                nc.tensor.matmul(
                    psr[:],
                    lhsT=aab[:, c, hp, :],
                    rhs=tTb[:, hp, nh * 512:(nh + 1) * 512],
                    start=True,
                    stop=True,
                )
                nc.scalar.activation(
                    out=srcb[:, hp, nh * 512:(nh + 1) * 512], in_=psr[:],
                    func=AF.Copy,
                )
            # srd write + head broadcasts as soon as this hp's rows exist
            nc.sync.dma_start(out=srd[c, :, hp, :], in_=srcb[:, hp, :])
            make_sbc(c * NH + 2 * hp)
            make_sbc(c * NH + 2 * hp + 1)

        # ---- A1: Hm with ones column -> H_aug[c] ----
        for nb in range(NB):
            ph = pshm.tile([128, 260], F32, tag="ph")
            nc.tensor.matmul(
                ph[:],
                lhsT=hTe[:, nb // 4, c, (nb % 4) * 128:(nb % 4 + 1) * 128],
                rhs=we[:, c, :],
                start=True,
                stop=True,
            )
            nc.scalar.activation(
                out=H_aug[:, c, nb, :, :],
                in_=ph[:].rearrange("p (h o) -> p h o", h=NH),
                func=AF.Copy,
            )

        # ---- A5: Hbd[c] = (b .* Haug | d .* Haug) ----
        def hbd_scale(engine, out, in_, col):
            eng[engine].tensor_tensor(
                out=out, in0=in_, in1=col.to_broadcast(in_.shape),
                op=OP.mult,
            )

        # batched over all 4 heads: col value broadcast along the 65-wide
        # free dim ([128, hp, hr, 1] -> [128, hp, hr, 65])
        for nb in range(NB):
            hbd_scale(
                HBD_B_ENGINE,
                Hbd[:, c, nb, :, 0:65].rearrange(
                    "p (hp hr) o -> p hp hr o", hp=2),
                H_aug[:, c, nb, :, :].rearrange(
                    "p (hp hr) o -> p hp hr o", hp=2),
                bcol[:, :, :, nb:nb + 1],
            )
            hbd_scale(
                HBD_D_ENGINE,
                Hbd[:, c, nb, :, 65:130].rearrange(
                    "p (hp hr) o -> p hp hr o", hp=2),
                H_aug[:, c, nb, :, :].rearrange(
                    "p (hp hr) o -> p hp hr o", hp=2),
                dcol[:, :, :, nb:nb + 1],
            )

    # =================== stage B (8 global heads) ===================
    def make_ov(c):
        # ovn = -(V @ (d .* Haug)) staged bf16 [128, ib, h, 65]
        for ib in range(NB):
            pv = psov.tile([128, 260], F32, tag="pv")
            for jb in range(NB):
                nc.tensor.matmul(
                    pv[:],
                    lhsT=vT[:, jb, ib * 128:(ib + 1) * 128],
                    rhs=Hbd[:, c, jb, :, 65:130],
                    start=(jb == 0),
                    stop=(jb == NB - 1),
                )
            nc.scalar.activation(
                out=ovn[:, c, ib, :, :],
                in_=pv[:].rearrange("p (h o) -> p h o", h=NH),
                func=AF.Copy,
                scale=-1.0,
            )

    def make_vpt(gh):
        c, h = gh // NH, gh % NH
        hp, hr = h // 2, h % 2
        sbc = sbc_l.pop(gh)
        VpT = vppool.tile([128, NB, N], BF16, tag="vpt")
        for jb in range(NB):
            dn = svl[:, c, hp, jb, 2 * hr + 1:2 * hr + 2]
            if jb >= NB - MASK_NG:
                # gpsimd cannot run STT; use an is_ge/mult TT pair
                m01 = sbcp.tile([128, N], BF16, tag="m01")
                nc.gpsimd.tensor_tensor(
                    out=m01[:], in0=sbc[:],
                    in1=dn.to_broadcast([128, N]), op=OP.is_ge,
                )
                nc.gpsimd.tensor_tensor(
                    out=VpT[:, jb, :], in0=m01[:], in1=vT[:, jb, :],
                    op=OP.mult,
                )
            else:
                nc.vector.scalar_tensor_tensor(
                    out=VpT[:, jb, :],
                    in0=sbc[:],
                    scalar=dn,
                    in1=vT[:, jb, :],
                    op0=OP.is_ge,
                    op1=OP.mult,
                )
        vpt_l[gh] = VpT

    make_ov(0)
    make_vpt(0)
    make_vpt(1)

    for gh in range(C2 * NH):
        c, h = gh // NH, gh % NH
        hp, hr = h // 2, h % 2
        if gh == 3:
            make_ov(1)
        VpT = vpt_l.pop(gh)
        # mmneg = -num
        mmneg = smallp.tile([128, NB, 65], F32, tag="mmneg")
        rec = vecp.tile([128, NB, 1], F32, tag="rec")
        stage = smallp.tile([128, NB, F], F32, tag="stage")
        for half in range(2):
            ppall = pspp.tile([128, 4, 256], F32, tag="pp")
            u_t = smallp.tile([128, 4, 65], F32, tag="u")
            for k in range(4):
                ib = half * 4 + k
                for jb in range(NB):
                    nc.tensor.matmul(
                        ppall[:, k, 0:130],
                        lhsT=VpT[:, jb, ib * 128:(ib + 1) * 128],
                        rhs=Hbd[:, c, jb, h, :],
                        start=(jb == 0),
                        stop=(jb == NB - 1),
                    )
                # inject -ov into the d-side columns while this slot's
                # accumulation group is still the bank's current one
                nc.tensor.matmul(
                    ppall[:, k, 65:130],
                    lhsT=ident[:],
                    rhs=ovn[:, c, ib, h, :],
                    start=False,
                    stop=True,
                    skip_group_check=True,
                )
            for k in range(4):
                ib = half * 4 + k
                # u[k] = r * (ppd - ov)   (ACT scaled copy, one PSUM read)
                nc.scalar.activation(
                    out=u_t[:, k, :], in_=ppall[:, k, 65:130], func=AF.Copy,
                    scale=rr[:, c, hp, hr, ib:ib + 1],
                )
            # -num = u - ppb   (one batched TT, one PSUM operand)
            nc.vector.tensor_tensor(
                out=mmneg[:, half * 4:half * 4 + 4, :],
                in0=u_t[:],
                in1=ppall[:, :, 0:65],
                op=OP.subtract,
            )
        if gh + 2 < 8:
            make_vpt(gh + 2)
        # rec = -1/den ; out = (-num[:, :64]) * rec = num/den
        for half in range(2):
            s4 = slice(half * 4, half * 4 + 4)
            nc.vector.reciprocal(out=rec[:, s4], in_=mmneg[:, s4, 64:65])
            nc.gpsimd.tensor_tensor(
                out=stage[:, s4],
                in0=mmneg[:, s4, 0:64],
                in1=rec[:, s4].to_broadcast([128, 4, F]),
                op=OP.mult,
            )
            nc.sync.dma_start(
                out=out_ap[c, h][half * 512:(half + 1) * 512, :].rearrange(
                    "(ib p) o -> p ib o", p=128),
                in_=stage[:, s4],
            )


def _install_ntff_hook():
    """antenv.axon_hooks is missing in this image; inject an equivalent shim
    driving NTFF profiling via ctypes into libaxon_pjrt.so."""
    import types, ctypes, contextlib

    if "antenv.axon_hooks" in sys.modules:
        return
    so_path = "/opt/axon/libaxon_pjrt.so"
    try:
        lib = ctypes.CDLL(so_path)
        lib.axon_start_nrt_profile.argtypes = [
            ctypes.POINTER(ctypes.c_int64),
            ctypes.c_size_t,
        ]
        lib.axon_start_nrt_profile.restype = ctypes.c_int64
        lib.axon_stop_nrt_profile.argtypes = [ctypes.c_char_p]
        lib.axon_stop_nrt_profile.restype = ctypes.c_int64
    except (OSError, AttributeError):
        return

    @contextlib.contextmanager
    def _hook(output_dir, device_ids):
        import jax

        jax.devices()
        if device_ids:
            ids = (ctypes.c_int64 * len(device_ids))(*device_ids)
            rc = lib.axon_start_nrt_profile(ids, len(device_ids))
        else:
            rc = lib.axon_start_nrt_profile(None, 0)
        if rc != 0:
            raise RuntimeError(f"axon_start_nrt_profile rc={rc}")
        try:
            yield
        finally:
            n = lib.axon_stop_nrt_profile(str(output_dir).encode())
            print(f"profile: {n} file(s) written to {output_dir}", file=sys.stderr)

    mod = types.ModuleType("antenv.axon_hooks")
    mod.get_axon_ntff_profile_hook = lambda: _hook
    mod.set_axon_ntff_profile_hook = lambda h: None
    sys.modules["antenv.axon_hooks"] = mod

    import concourse.bass_utils as bu

    bu.upload_artifacts = lambda tmpdir: f"local:{tmpdir}"


_CACHED = {}


def _build_program():
    if "nc" in _CACHED:
        return _CACHED["nc"]
    nc = bacc.Bacc(
        "TRN2",
        target_bir_lowering=False,
        debug=False,
        enable_asserts=True,
        num_devices=8,
    )
    ins = {
        "vT": nc.dram_tensor("vT", [N, N], BF16, kind="ExternalInput").ap(),
        "hTe": nc.dram_tensor(
            "hTe", [2, 65, C2, 512], BF16, kind="ExternalInput"
        ).ap(),
        "we": nc.dram_tensor(
            "we", [65, C2, NH * 65], BF16, kind="ExternalInput"
        ).ap(),
        "wb": nc.dram_tensor("wb", [64, C2, NH, F], BF16, kind="ExternalInput").ap(),
        "aab": nc.dram_tensor("aab", [128, C2, 2, 4], BF16, kind="ExternalInput").ap(),
    }
    out_ap = nc.dram_tensor(
        "out_loc", [C2, NH, N, F], F32, kind="ExternalOutput"
    ).ap()
    with tile.TileContext(nc) as tc:
        with ExitStack() as ctx:
            build_kernel(nc, tc, ctx, ins, out_ap)
    nc.compile()
    _CACHED["nc"] = nc
    return nc


def make_in_maps(h, adj, w, a_src, a_dst):
    bf = ml_dtypes.bfloat16
    eye = np.eye(N, dtype=np.float32)
    in_maps = []
    for core in range(8):
        b, cp = core // 2, core % 2
        cs = slice(2 * cp, 2 * cp + 2)
        # vT[j, i] = 1{adj[b][i, j] or i == j}
        vT = (((adj[b] + eye) > 0).astype(np.float32).T).astype(bf)
        # hTe: [nh, 65, 2, 512]; rows 0:64 = h[b, c].T, row 64 = ones
        hTe0 = np.zeros((65, 2, N), np.float32)
        hTe0[0:64] = h[b, cs].transpose(2, 0, 1)
        hTe0[64] = 1.0
        hTe = np.ascontiguousarray(
            hTe0.reshape(65, 2, 2, 512).transpose(2, 0, 1, 3))
        # we: [65, 2, 4*65]: per head block 65 cols: w | e65
        we = np.zeros((65, 2, NH * 65), np.float32)
        for ci in range(2):
            for hh in range(NH):
                we[0:64, ci, hh * 65:hh * 65 + 64] = w[2 * cp + ci, hh]
                we[64, ci, hh * 65 + 64] = 1.0
        wv = np.ascontiguousarray(w[cs].transpose(2, 0, 1, 3))  # [64,2,4,64]
        # aab: [128, 2, 2, 4] block-diag (src_h0, -dst_h0, src_h1, -dst_h1)
        aab = np.zeros((128, 2, 2, 4), np.float32)
        for ci in range(2):
            for hp in range(2):
                aab[0:64, ci, hp, 0] = a_src[2 * cp + ci, 2 * hp, :, 0]
                aab[0:64, ci, hp, 1] = -a_dst[2 * cp + ci, 2 * hp, :, 0]
                aab[64:128, ci, hp, 2] = a_src[2 * cp + ci, 2 * hp + 1, :, 0]
                aab[64:128, ci, hp, 3] = -a_dst[2 * cp + ci, 2 * hp + 1, :, 0]
        in_maps.append(
            {
                "vT": np.ascontiguousarray(vT),
                "hTe": hTe.astype(bf),
                "we": we.astype(bf),
                "wb": wv.astype(bf),
                "aab": aab.astype(bf),
            }
        )
    return in_maps


def kernel(h, adj, w, a_src, a_dst, trace=False):
    h = np.asarray(h, np.float32)
    adj = np.asarray(adj, np.float32)
    w = np.asarray(w, np.float32)
    a_src = np.asarray(a_src, np.float32)
    a_dst = np.asarray(a_dst, np.float32)
    nc = _build_program()
    in_maps = make_in_maps(h, adj, w, a_src, a_dst)
    if trace:
        _install_ntff_hook()
    res = run_bass_kernel_spmd(nc, in_maps, list(range(8)), trace=trace)
    out = np.zeros((4, 4, 4, N, F), np.float32)
    for core in range(8):
        b, cp = core // 2, core % 2
        out[b, 2 * cp:2 * cp + 2] = res.results[core]["out_loc"]
    if trace:
        return out, res
    return out


# revision 60
# speedup vs baseline: 1.0454x; 1.0454x over previous
"""Trainium2 Bass kernel for nn_BatchMultiHeadGraphAttention (v3).

Math: out[b,c,h] = softmax_j(mask(leaky(src_i + dst_j))) @ Hm  where
Hm = h[b,c] @ w[c,h], t = tanh(Hm), src = t @ a_src, dst = t @ a_dst.

Identity: exp(leaky(x)) = max(e^x, e^{0.2x}), both branches rank-1 in
(i,j).  With s_ij = 1{src_i >= -dst_j}, Vp = V .* s (V = adj+selfloops):
  num = Vp @ (b .* Haug) + r_i * ( V @ (d .* Haug) - Vp @ (d .* Haug) )
  b_j = e^{dst_j}, d_j = e^{0.2 dst_j}, r_i = e^{-0.8 src_i},
  Haug = [Hm | 1];  out = num[:, :64] / num[:, 64].

v6 (132.9us, from 144.9us baseline): negated-ov injected into the
d-columns of the Vp PSUM while each slot's accumulation group is still
the bank's current one (PSUM accumulate groups are per-bank: an
out-of-group start=False matmul OVERWRITES), killing the rov staging;
combine = per-ib ACT u=r*(ppd-ov) + one batched vector TT per
half-head over a 4-slot padded PSUM tile (-num; sign cancels in
num/den); batched attention-vector exps via strided APs; Hbd scaled
via broadcast TTs on gpsimd batched over heads; depth-2 mask prefetch
interleaved with the combine TTs on the in-order vector queue;
per-half reciprocal/divide/DMA drain.

Engine constraints learned: Pool(gpsimd) has no TensorScalarPtr ops,
no PSUM access, no is_ge; DVE STT supports no 2x perf modes (4 memory
streams); tensor_mask/tensor_paged_mask stock encoders crash the
device; DVE ops can read at most one PSUM operand.

Sharding: core = b*2 + cpair; each core does one b and two c's (4 heads).
"""

import os
import sys
from contextlib import ExitStack

import numpy as np
import ml_dtypes

sys.path.insert(0, "/opt/trn_rl_repo")

import concourse.bass as bass
import concourse.bacc as bacc
import concourse.tile as tile
from concourse import mybir
from concourse.masks import make_identity
from concourse.bass_utils import run_bass_kernel_spmd

F32 = mybir.dt.float32
BF16 = mybir.dt.bfloat16
AF = mybir.ActivationFunctionType
OP = mybir.AluOpType

N = 1024
NB = 8
F = 64
C2 = 2
NH = 4

# tuning knobs (gpsimd/Pool cannot run TensorScalarPtr ops: no TSP/STT there)
MASK_NG = int(os.environ.get("MASK_NG", "0"))      # jb's per head on gpsimd
COMBINE_ENGINE = os.environ.get("COMBINE_ENGINE", "gpsimd")
STAGE_ENGINE = os.environ.get("STAGE_ENGINE", "gpsimd")
HBD_B_ENGINE = os.environ.get("HBD_B_ENGINE", "gpsimd")
HBD_D_ENGINE = os.environ.get("HBD_D_ENGINE", "gpsimd")


def build_kernel(nc: bass.Bass, tc: tile.TileContext, ctx: ExitStack, ins, out_ap):
    vT_ap = ins["vT"]
    hTe_ap = ins["hTe"]
    we_ap = ins["we"]
    wb_ap = ins["wb"]
    aab_ap = ins["aab"]

    # ---------------- pools ----------------
    constp = ctx.enter_context(tc.tile_pool(name="const", bufs=1))
    apool = ctx.enter_context(tc.tile_pool(name="apool", bufs=1))
    vppool = ctx.enter_context(tc.tile_pool(name="vppool", bufs=4))
    smallp = ctx.enter_context(tc.tile_pool(name="smallp", bufs=2))
    vecp = ctx.enter_context(tc.tile_pool(name="vecp", bufs=2))
    sbcp = ctx.enter_context(tc.tile_pool(name="sbcp", bufs=8))
    pshm = ctx.enter_context(tc.tile_pool(name="pshm", bufs=2, space="PSUM"))
    psvec = ctx.enter_context(tc.tile_pool(name="psvec", bufs=1, space="PSUM"))
    psov = ctx.enter_context(tc.tile_pool(name="psov", bufs=1, space="PSUM"))
    # half-head PSUM: 4 ib-slots of 256 f32 (padded to avoid bank straddle)
    pspp = ctx.enter_context(tc.tile_pool(name="pspp", bufs=2, space="PSUM"))
    dramp = ctx.enter_context(tc.tile_pool(name="dramp", bufs=1, space="DRAM"))

    # ---------------- constants (small params first, vT after) ----------
    vT = constp.tile([128, NB, N], BF16)
    hTe = constp.tile([65, 2, C2, 512], BF16)  # [65, nh, c, 512]
    wb = constp.tile([64, C2, NH, F], BF16)
    aab = constp.tile([128, C2, 2, 4], BF16)
    we = constp.tile([65, C2, NH * 65], BF16)
    nc.sync.dma_start(out=wb[:], in_=wb_ap[:])
    nc.sync.dma_start(out=hTe[:, 0], in_=hTe_ap[0])
    nc.sync.dma_start(out=hTe[:, 1], in_=hTe_ap[1])
    nc.sync.dma_start(out=aab[:], in_=aab_ap[:])
    nc.sync.dma_start(out=we[:], in_=we_ap[:])
    for jb in range(NB):
        nc.sync.dma_start(out=vT[:, jb, :], in_=vT_ap[jb * 128:(jb + 1) * 128, :])

    # ---------------- persistent A-stage outputs ----------------
    H_aug = apool.tile([128, C2, NB, NH, 65], BF16)
    Hbd = apool.tile([128, C2, NB, NH, 130], BF16)
    srcb_l = [apool.tile([4, 2, N], BF16, tag=f"srcb{i}", name=f"srcb{i}")
              for i in range(C2)]
    srd = dramp.tile([C2, 4, 2, N], BF16, tag="srd")
    # svl[:, c, hp, nb, k]: k = (src_h0, negdst_h0, src_h1, negdst_h1)
    svl = apool.tile([128, C2, 2, NB, 4], F32, tag="svl")
    # rr[:, c, hp, hr, nb] = e^{-0.8 src} for head h = 2*hp + hr
    rr = apool.tile([128, C2, 2, 2, NB], F32, tag="rr")
    # ovn[:, c, ib, h, :] = -(V @ (d .* Haug)) staged bf16
    ovn = apool.tile([128, C2, NB, NH, 65], BF16, tag="ovn")
    ident = constp.tile([128, 128], BF16)
    make_identity(nc, ident[:])

    sbc_l = {}
    vpt_l = {}

    def make_sbc(gh):
        c, h = gh // NH, gh % NH
        hp, hr = h // 2, h % 2
        sbc = sbcp.tile([128, N], BF16, tag="sbc")
        nc.sync.dma_start(
            out=sbc[:],
            in_=srd[c, 2 * hr:2 * hr + 1, hp, :].to_broadcast([128, N]),
        )
        sbc_l[gh] = sbc

    eng = {"vector": nc.vector, "gpsimd": nc.gpsimd}

    # =================== stage A (both c) ===================
    for c in range(C2):
        # ---- A2: tTb = tanh(Hm).T per head pair [128, hp, N] bf16 ----
        tTb = smallp.tile([128, 2, N], BF16, tag="ttb")
        for hp in range(2):
            for nh in range(2):
                pht = pshm.tile([128, 512], F32, tag="ph")
                nc.tensor.matmul(
                    pht[:],
                    lhsT=wb[:, c, 2 * hp:2 * hp + 2, :],
                    rhs=hTe[0:64, nh, c, :],
                    start=True,
                    stop=True,
                )
                nc.scalar.activation(
                    out=tTb[:, hp, nh * 512:(nh + 1) * 512], in_=pht[:],
                    func=AF.Tanh,
                )

        # ---- A3: attention vectors in column layout ----
        psv = psvec.tile([128, 2, NB, 4], F32, tag="psv")
        for hp in range(2):
            for nb in range(NB):
                nc.tensor.matmul(
                    psv[:, hp, nb, :],
                    lhsT=tTb[:, hp, nb * 128:(nb + 1) * 128],
                    rhs=aab[:, c, hp, :],
                    start=True,
                    stop=True,
                )
        nc.scalar.activation(out=svl[:, c], in_=psv[:], func=AF.Copy)

        # batched per-head exps; (hp, hr, nb) AP order == h-major layout
        sv_k = svl[:, c].rearrange("p hp nb (hr two) -> p hp hr nb two", hr=2)
        src_ap = sv_k[:, :, :, :, 0]
        dst_ap = sv_k[:, :, :, :, 1]
        nc.scalar.activation(
            out=rr[:, c], in_=src_ap, func=AF.Exp, scale=-0.8,
        )
        bcol = vecp.tile([128, 2, 2, NB], F32, tag="bcol")
        dcol = vecp.tile([128, 2, 2, NB], F32, tag="dcol")
        nc.scalar.activation(out=bcol[:], in_=dst_ap, func=AF.Exp, scale=-1.0)
        nc.scalar.activation(out=dcol[:], in_=dst_ap, func=AF.Exp, scale=-0.2)

        # ---- A4: src row layout -> DRAM (for free-dim broadcast) ----
        srcb = srcb_l[c]
        for hp in range(2):
            for nh in range(2):
                psr = psvec.tile([4, 512], F32, tag="psv")
                nc.tensor.matmul(
                    psr[:],
                    lhsT=aab[:, c, hp, :],
                    rhs=tTb[:, hp, nh * 512:(nh + 1) * 512],
                    start=True,
                    stop=True,
                )
                nc.scalar.activation(
                    out=srcb[:, hp, nh * 512:(nh + 1) * 512], in_=psr[:],
                    func=AF.Copy,
                )
            # srd write + head broadcasts as soon as this hp's rows exist
            nc.sync.dma_start(out=srd[c, :, hp, :], in_=srcb[:, hp, :])
            make_sbc(c * NH + 2 * hp)
            make_sbc(c * NH + 2 * hp + 1)

        # ---- A1: Hm with ones column -> H_aug[c] ----
        for nb in range(NB):
            ph = pshm.tile([128, 260], F32, tag="ph")
            nc.tensor.matmul(
                ph[:],
                lhsT=hTe[:, nb // 4, c, (nb % 4) * 128:(nb % 4 + 1) * 128],
                rhs=we[:, c, :],
                start=True,
                stop=True,
            )
            nc.scalar.activation(
                out=H_aug[:, c, nb, :, :],
                in_=ph[:].rearrange("p (h o) -> p h o", h=NH),
                func=AF.Copy,
            )

        # ---- A5: Hbd[c] = (b .* Haug | d .* Haug) ----
        def hbd_scale(engine, out, in_, col):
            eng[engine].tensor_tensor(
                out=out, in0=in_, in1=col.to_broadcast(in_.shape),
                op=OP.mult,
            )

        # batched over all 4 heads: col value broadcast along the 65-wide
        # free dim ([128, hp, hr, 1] -> [128, hp, hr, 65])
        for nb in range(NB):
            hbd_scale(
                HBD_B_ENGINE,
                Hbd[:, c, nb, :, 0:65].rearrange(
                    "p (hp hr) o -> p hp hr o", hp=2),
                H_aug[:, c, nb, :, :].rearrange(
                    "p (hp hr) o -> p hp hr o", hp=2),
                bcol[:, :, :, nb:nb + 1],
            )
            hbd_scale(
                HBD_D_ENGINE,
                Hbd[:, c, nb, :, 65:130].rearrange(
                    "p (hp hr) o -> p hp hr o", hp=2),
                H_aug[:, c, nb, :, :].rearrange(
                    "p (hp hr) o -> p hp hr o", hp=2),
                dcol[:, :, :, nb:nb + 1],
            )

    # =================== stage B (8 global heads) ===================
    def make_ov(c):
        # ovn = -(V @ (d .* Haug)) staged bf16 [128, ib, h, 65]
        for ib in range(NB):
            pv = psov.tile([128, 260], F32, tag="pv")
            for jb in range(NB):
                nc.tensor.matmul(
                    pv[:],
                    lhsT=vT[:, jb, ib * 128:(ib + 1) * 128],
                    rhs=Hbd[:, c, jb, :, 65:130],
                    start=(jb == 0),
                    stop=(jb == NB - 1),
                )
            nc.scalar.activation(
                out=ovn[:, c, ib, :, :],
                in_=pv[:].rearrange("p (h o) -> p h o", h=NH),
                func=AF.Copy,
                scale=-1.0,
            )

    def make_vpt(gh):
        c, h = gh // NH, gh % NH
        hp, hr = h // 2, h % 2
        sbc = sbc_l.pop(gh)
        VpT = vppool.tile([128, NB, N], BF16, tag="vpt")
        for jb in range(NB):
            dn = svl[:, c, hp, jb, 2 * hr + 1:2 * hr + 2]
            if jb >= NB - MASK_NG:
                # gpsimd cannot run STT; use an is_ge/mult TT pair
                m01 = sbcp.tile([128, N], BF16, tag="m01")
                nc.gpsimd.tensor_tensor(
                    out=m01[:], in0=sbc[:],
                    in1=dn.to_broadcast([128, N]), op=OP.is_ge,
                )
                nc.gpsimd.tensor_tensor(
                    out=VpT[:, jb, :], in0=m01[:], in1=vT[:, jb, :],
                    op=OP.mult,
                )
            else:
                nc.vector.scalar_tensor_tensor(
                    out=VpT[:, jb, :],
                    in0=sbc[:],
                    scalar=dn,
                    in1=vT[:, jb, :],
                    op0=OP.is_ge,
                    op1=OP.mult,
                )
        vpt_l[gh] = VpT

    make_ov(0)
    make_vpt(0)
    make_vpt(1)

    for gh in range(C2 * NH):
        c, h = gh // NH, gh % NH
        hp, hr = h // 2, h % 2
        if gh == 3:
            make_ov(1)
        VpT = vpt_l.pop(gh)
        # mmneg = -num
        mmneg = smallp.tile([128, NB, 65], F32, tag="mmneg")
        rec = vecp.tile([128, NB, 1], F32, tag="rec")
        stage = smallp.tile([128, NB, F], F32, tag="stage")
        for half in range(2):
            ppall = pspp.tile([128, 4, 256], F32, tag="pp")
            u_t = smallp.tile([128, 4, 65], F32, tag="u")
            for k in range(4):
                ib = half * 4 + k
                for jb in range(NB):
                    nc.tensor.matmul(
                        ppall[:, k, 0:130],
                        lhsT=VpT[:, jb, ib * 128:(ib + 1) * 128],
                        rhs=Hbd[:, c, jb, h, :],
                        start=(jb == 0),
                        stop=(jb == NB - 1),
                    )
                # inject -ov into the d-side columns while this slot's
                # accumulation group is still the bank's current one
                nc.tensor.matmul(
                    ppall[:, k, 65:130],
                    lhsT=ident[:],
                    rhs=ovn[:, c, ib, h, :],
                    start=False,
                    stop=True,
                    skip_group_check=True,
                )
            for k in range(4):
                ib = half * 4 + k
                # u[k] = r * (ppd - ov)   (ACT scaled copy, one PSUM read)
                nc.scalar.activation(
                    out=u_t[:, k, :], in_=ppall[:, k, 65:130], func=AF.Copy,
                    scale=rr[:, c, hp, hr, ib:ib + 1],
                )
            # -num = u - ppb   (one batched TT, one PSUM operand)
            nc.vector.tensor_tensor(
                out=mmneg[:, half * 4:half * 4 + 4, :],
                in0=u_t[:],
                in1=ppall[:, :, 0:65],
                op=OP.subtract,
            )
        if gh + 2 < 8:
            make_vpt(gh + 2)
        # rec = -1/den ; out = (-num[:, :64]) * rec = num/den
        for half in range(2):
            s4 = slice(half * 4, half * 4 + 4)
            nc.vector.reciprocal(out=rec[:, s4], in_=mmneg[:, s4, 64:65])
            nc.gpsimd.tensor_tensor(
                out=stage[:, s4],
                in0=mmneg[:, s4, 0:64],
                in1=rec[:, s4].to_broadcast([128, 4, F]),
                op=OP.mult,
            )
            nc.sync.dma_start(
                out=out_ap[c, h][half * 512:(half + 1) * 512, :].rearrange(
                    "(ib p) o -> p ib o", p=128),
                in_=stage[:, s4],
            )


def _install_ntff_hook():
    """antenv.axon_hooks is missing in this image; inject an equivalent shim
    driving NTFF profiling via ctypes into libaxon_pjrt.so."""
    import types, ctypes, contextlib

    if "antenv.axon_hooks" in sys.modules:
        return
    so_path = "/opt/axon/libaxon_pjrt.so"
    try:
        lib = ctypes.CDLL(so_path)
        lib.axon_start_nrt_profile.argtypes = [
            ctypes.POINTER(ctypes.c_int64),
            ctypes.c_size_t,
        ]
        lib.axon_start_nrt_profile.restype = ctypes.c_int64
        lib.axon_stop_nrt_profile.argtypes = [ctypes.c_char_p]
        lib.axon_stop_nrt_profile.restype = ctypes.c_int64
    except (OSError, AttributeError):
        return

    @contextlib.contextmanager
    def _hook(output_dir, device_ids):
        import jax

        jax.devices()
        if device_ids:
            ids = (ctypes.c_int64 * len(device_ids))(*device_ids)
            rc = lib.axon_start_nrt_profile(ids, len(device_ids))
        else:
            rc = lib.axon_start_nrt_profile(None, 0)
        if rc != 0:
            raise RuntimeError(f"axon_start_nrt_profile rc={rc}")
        try:
            yield
        finally:
            n = lib.axon_stop_nrt_profile(str(output_dir).encode())
            print(f"profile: {n} file(s) written to {output_dir}", file=sys.stderr)

    mod = types.ModuleType("antenv.axon_hooks")
    mod.get_axon_ntff_profile_hook = lambda: _hook
    mod.set_axon_ntff_profile_hook = lambda h: None
    sys.modules["antenv.axon_hooks"] = mod

    import concourse.bass_utils as bu

    bu.upload_artifacts = lambda tmpdir: f"local:{tmpdir}"


_CACHED = {}


def _build_program():
    if "nc" in _CACHED:
        return _CACHED["nc"]
    nc = bacc.Bacc(
        "TRN2",
        target_bir_lowering=False,
        debug=False,
        enable_asserts=True,
        num_devices=8,
    )
    ins = {
        "vT": nc.dram_tensor("vT", [N, N], BF16, kind="ExternalInput").ap(),
        "hTe": nc.dram_tensor(
            "hTe", [2, 65, C2, 512], BF16, kind="ExternalInput"
        ).ap(),
        "we": nc.dram_tensor(
            "we", [65, C2, NH * 65], BF16, kind="ExternalInput"
        ).ap(),
        "wb": nc.dram_tensor("wb", [64, C2, NH, F], BF16, kind="ExternalInput").ap(),
        "aab": nc.dram_tensor("aab", [128, C2, 2, 4], BF16, kind="ExternalInput").ap(),
    }
    out_ap = nc.dram_tensor(
        "out_loc", [C2, NH, N, F], F32, kind="ExternalOutput"
    ).ap()
    with tile.TileContext(nc) as tc:
        with ExitStack() as ctx:
            build_kernel(nc, tc, ctx, ins, out_ap)
    nc.compile()
    _CACHED["nc"] = nc
    return nc


def make_in_maps(h, adj, w, a_src, a_dst):
    bf = ml_dtypes.bfloat16
    eye = np.eye(N, dtype=np.float32)
    in_maps = []
    for core in range(8):
        b, cp = core // 2, core % 2
        cs = slice(2 * cp, 2 * cp + 2)
        # vT[j, i] = 1{adj[b][i, j] or i == j}
        vT = (((adj[b] + eye) > 0).astype(np.float32).T).astype(bf)
        # hTe: [nh, 65, 2, 512]; rows 0:64 = h[b, c].T, row 64 = ones
        hTe0 = np.zeros((65, 2, N), np.float32)
        hTe0[0:64] = h[b, cs].transpose(2, 0, 1)
        hTe0[64] = 1.0
        hTe = np.ascontiguousarray(
            hTe0.reshape(65, 2, 2, 512).transpose(2, 0, 1, 3))
        # we: [65, 2, 4*65]: per head block 65 cols: w | e65
        we = np.zeros((65, 2, NH * 65), np.float32)
        for ci in range(2):
            for hh in range(NH):
                we[0:64, ci, hh * 65:hh * 65 + 64] = w[2 * cp + ci, hh]
                we[64, ci, hh * 65 + 64] = 1.0
        wv = np.ascontiguousarray(w[cs].transpose(2, 0, 1, 3))  # [64,2,4,64]
        # aab: [128, 2, 2, 4] block-diag (src_h0, -dst_h0, src_h1, -dst_h1)
        aab = np.zeros((128, 2, 2, 4), np.float32)
        for ci in range(2):
            for hp in range(2):
                aab[0:64, ci, hp, 0] = a_src[2 * cp + ci, 2 * hp, :, 0]
                aab[0:64, ci, hp, 1] = -a_dst[2 * cp + ci, 2 * hp, :, 0]
                aab[64:128, ci, hp, 2] = a_src[2 * cp + ci, 2 * hp + 1, :, 0]
                aab[64:128, ci, hp, 3] = -a_dst[2 * cp + ci, 2 * hp + 1, :, 0]
        in_maps.append(
            {
                "vT": np.ascontiguousarray(vT),
                "hTe": hTe.astype(bf),
                "we": we.astype(bf),
                "wb": wv.astype(bf),
                "aab": aab.astype(bf),
            }
        )
    return in_maps


def kernel(h, adj, w, a_src, a_dst, trace=False):
    h = np.asarray(h, np.float32)
    adj = np.asarray(adj, np.float32)
    w = np.asarray(w, np.float32)
    a_src = np.asarray(a_src, np.float32)
    a_dst = np.asarray(a_dst, np.float32)
    nc = _build_program()
    in_maps = make_in_maps(h, adj, w, a_src, a_dst)
    if trace:
        _install_ntff_hook()
    res = run_bass_kernel_spmd(nc, in_maps, list(range(8)), trace=trace)
    out = np.zeros((4, 4, 4, N, F), np.float32)
    for core in range(8):
        b, cp = core // 2, core % 2
        out[b, 2 * cp:2 * cp + 2] = res.results[core]["out_loc"]
    if trace:
        return out, res
    return out
